# revision 24
# baseline (speedup 1.0000x reference)
"""CAPS attention Trainium2 kernel.

Self-contained: kernel(**inputs) -> np.ndarray, full (unsharded) in/out.
Shards 16 heads x 2 batches over 8 NeuronCores (2 heads, both batches per
core). W_q/k/p/gate/clock columns are tensor-parallel, W_c rows are
row-parallel; the host sums the 8 partial outputs.

Precision plan (rel-err budget ~1e-2): q/k/gate projections need ~17 bits
through the cumsum/softmax winner-selection chain -> 3-pass bf16 hi/lo.
p is insensitive -> 1-pass bf16. v and clock -> 1-pass f32r. Branch
tensors and scores -> f32r (3 matmul passes). Attention probs + v ->
bf16. Out projection -> f32r.
"""

from contextlib import ExitStack

import numpy as np

import concourse.bass as bass  # noqa: F401  (bass types via bacc)
import concourse.tile as tile
from concourse import mybir, bacc
from concourse.bass_utils import run_bass_kernel_spmd

B, T, D, H = 2, 2048, 2048, 16
DH = 128          # head dim
HC = 2            # heads per core
C = HC * DH       # per-core channels = 256
NCORES = 8
TCH = 512         # t-chunk for projection phase
NCH = T // TCH
NTB = T // 128    # 16 t-sub-blocks
SCALE = float(1.0 / np.sqrt(np.float32(3 * DH)))

F32 = mybir.dt.float32
F32R = mybir.dt.float32r
BF16 = mybir.dt.bfloat16
AF = mybir.ActivationFunctionType
ALU = mybir.AluOpType
AX = mybir.AxisListType
SWAP32 = [(i + 1 if i % 2 == 0 else i - 1) for i in range(32)]


def _build_program():
    nc = bacc.Bacc(trn_type="TRN2")

    x_d = nc.dram_tensor("x", [B, T, D], F32, kind="ExternalInput")
    wq_d = nc.dram_tensor("wq", [D, C], F32, kind="ExternalInput")
    wk_d = nc.dram_tensor("wk", [D, C], F32, kind="ExternalInput")
    wg_d = nc.dram_tensor("wg", [D, C + HC], F32, kind="ExternalInput")
    wp_d = nc.dram_tensor("wp", [D, C], F32, kind="ExternalInput")
    wv_d = nc.dram_tensor("wv", [D, C], F32, kind="ExternalInput")
    wc_d = nc.dram_tensor("wc", [C, D], F32, kind="ExternalInput")
    cos_d = nc.dram_tensor("cosT", [DH, T], F32, kind="ExternalInput")
    sin_d = nc.dram_tensor("sinT", [DH, T], F32, kind="ExternalInput")
    sgn_d = nc.dram_tensor("sgn", [DH, 1], F32, kind="ExternalInput")
    id_d = nc.dram_tensor("ident", [128, 128], F32, kind="ExternalInput")
    out_d = nc.dram_tensor("out", [B, T, D], BF16, kind="ExternalOutput")

    sp = {}
    for nm in ["q", "k", "g", "p"]:
        sp[nm] = nc.dram_tensor(f"sp_{nm}", [B, HC, DH, T], F32, kind="Internal")
    sp_v = nc.dram_tensor("sp_v", [B, 128, NTB, C], BF16, kind="Internal")
    clk_d = nc.dram_tensor("sp_clk", [B, HC, T], F32, kind="Internal")

    with tile.TileContext(nc) as tc:
        with ExitStack() as ctx:
            perm = ctx.enter_context(tc.tile_pool(name="perm", bufs=1))

            id_t = perm.tile([128, 128], F32, tag="id_t")
            nc.sync.dma_start(id_t[:], id_d[:])
            id_b = perm.tile([128, 128], BF16, tag="id_b")
            nc.vector.tensor_copy(id_b[:], id_t[:])


            # ================= P1: projections =================
            with ExitStack() as p1:
                wp_ = p1.enter_context(tc.tile_pool(name="wpool", bufs=1))
                tp_ = p1.enter_context(tc.tile_pool(name="p1t", bufs=2))
                tr_ = p1.enter_context(tc.tile_pool(name="p1r", bufs=1))
                psum = p1.enter_context(
                    tc.tile_pool(name="psum1", bufs=2, space="PSUM"))

                def stage_half(wd, cols, koff):
                    stg = wp_.tile([128, 8, C + HC], F32, tag="w_stage")
                    s = stg[:, :, :cols]
                    nc.sync.dma_start(
                        s, wd[:, :].rearrange("(o i) c -> i o c", i=128)
                        [:, koff:koff + 8, :])
                    return stg

                def split_bf16(wd, cols, pfx):
                    hi = wp_.tile([128, 16, cols], BF16, tag=f"{pfx}_hi",
                                  name=f"{pfx}_hi")
                    lo = wp_.tile([128, 16, cols], BF16, tag=f"{pfx}_lo",
                                  name=f"{pfx}_lo")
                    for koff in (0, 8):
                        stg = stage_half(wd, cols, koff)
                        s = stg[:, :, :cols]
                        hsl = hi[:, koff:koff + 8, :]
                        nc.vector.tensor_copy(hsl, s)
                        nc.vector.tensor_sub(lo[:, koff:koff + 8, :], s, hsl)
                    return hi, lo

                w_q_hi, w_q_lo = split_bf16(wq_d, C, "wq")
                w_k_hi, w_k_lo = split_bf16(wk_d, C, "wk")
                # gate: bf16 hi/lo on gate cols; clock cols as f32r
                w_g_hi = wp_.tile([128, 16, C], BF16, tag="wg_hi")
                w_g_lo = wp_.tile([128, 16, C], BF16, tag="wg_lo")
                w_clk = wp_.tile([128, 16, HC], F32R, tag="w_clk")
                for koff in (0, 8):
                    stg = stage_half(wg_d, C + HC, koff)
                    hsl = w_g_hi[:, koff:koff + 8, :]
                    nc.vector.tensor_copy(hsl, stg[:, :, :C])
                    nc.vector.tensor_sub(w_g_lo[:, koff:koff + 8, :],
                                         stg[:, :, :C], hsl)
                    nc.vector.tensor_copy(w_clk[:, koff:koff + 8, :],
                                          stg[:, :, C:C + HC])
                w_p = wp_.tile([128, 16, C], BF16, tag="w_p")
                for koff in (0, 8):
                    stg = stage_half(wp_d, C, koff)
                    nc.vector.tensor_copy(w_p[:, koff:koff + 8, :],
                                          stg[:, :, :C])
                w_v = wp_.tile([128, 16, C], F32R, tag="w_v")
                for koff in (0, 8):
                    stg = stage_half(wv_d, C, koff)
                    nc.vector.tensor_copy(w_v[:, koff:koff + 8, :],
                                          stg[:, :, :C])

                for b in range(B):
                    for chk in range(NCH):
                        xTr = tr_.tile([128, 16, TCH], F32R, tag="xTr")
                        xhb = tp_.tile([128, 16, TCH], BF16, tag="xhb")
                        xlb = tp_.tile([128, 16, TCH], BF16, tag="xlb")
                        for ts in range(TCH // 128):
                            r0 = chk * TCH + ts * 128
                            for xh in range(2):
                                xt = tp_.tile([128, 1024], F32, tag="x_in")
                                nc.sync.dma_start(
                                    xt[:], x_d[b, r0:r0 + 128,
                                               xh * 1024:(xh + 1) * 1024])
                                for kg2 in range(2):
                                    kg = xh * 2 + kg2
                                    pst = psum.tile([128, 512], F32,
                                                    tag="ps_tr")
                                    for j in range(4):
                                        ko = kg2 * 4 + j
                                        nc.tensor.transpose(
                                            pst[:, j * 128:(j + 1) * 128],
                                            xt[:, ko * 128:(ko + 1) * 128],
                                            id_t[:])
                                    pr = pst[:].rearrange(
                                        "p (a b) -> p a b", a=4)
                                    ksl = slice(kg * 4, (kg + 1) * 4)
                                    xsl = slice(ts * 128, (ts + 1) * 128)
                                    nc.scalar.copy(xhb[:, ksl, xsl], pr)
                                    nc.vector.tensor_sub(xlb[:, ksl, xsl], pr,
                                                         xhb[:, ksl, xsl])
                                    nc.vector.tensor_copy(xTr[:, ksl, xsl],
                                                          pr)

                        tsl = slice(chk * TCH, (chk + 1) * TCH)
                        # q,k,g: 3-pass bf16 hi/lo
                        for nm, whi, wlo in [("q", w_q_hi, w_q_lo),
                                             ("k", w_k_hi, w_k_lo),
                                             ("g", w_g_hi, w_g_lo)]:
                            for h in range(HC):
                                hsl = slice(h * DH, (h + 1) * DH)
                                ps = psum.tile([128, TCH], F32, tag="ps_proj")
                                for ko in range(16):
                                    nc.tensor.matmul(
                                        ps[:], whi[:, ko, hsl], xhb[:, ko, :],
                                        start=(ko == 0), stop=False)
                                for ko in range(16):
                                    nc.tensor.matmul(
                                        ps[:], wlo[:, ko, hsl], xhb[:, ko, :],
                                        start=False, stop=False)
                                for ko in range(16):
                                    nc.tensor.matmul(
                                        ps[:], whi[:, ko, hsl], xlb[:, ko, :],
                                        start=False, stop=(ko == 15))
                                prc = tp_.tile([128, TCH], F32, tag="prcp")
                                nc.scalar.copy(prc[:], ps[:])
                                nc.sync.dma_start(sp[nm][b, h, :, tsl], prc[:])
                        # p: 1-pass bf16
                        for h in range(HC):
                            ps = psum.tile([128, TCH], F32, tag="ps_proj")
                            for ko in range(16):
                                nc.tensor.matmul(
                                    ps[:], w_p[:, ko, h * DH:(h + 1) * DH],
                                    xhb[:, ko, :],
                                    start=(ko == 0), stop=(ko == 15))
                            prc = tp_.tile([128, TCH], F32, tag="prcp")
                            nc.scalar.copy(prc[:], ps[:])
                            nc.sync.dma_start(sp["p"][b, h, :, tsl], prc[:])
                        # clock: 1-pass f32r, weight-stationary -> [HC, TCH]
                        psc = psum.tile([HC, TCH], F32, tag="ps_clk")
                        for ko in range(16):
                            nc.tensor.matmul(
                                psc[:], w_clk[:, ko, :], xTr[:, ko, :],
                                start=(ko == 0), stop=(ko == 15))
                        ccp = tp_.tile([HC, TCH], F32, tag="clkcp")
                        nc.scalar.copy(ccp[:], psc[:])
                        nc.sync.dma_start(clk_d[b, :, tsl], ccp[:])
                        # v: 1-pass f32r -> [t, C], spilled bf16
                        for ts in range(TCH // 128):
                            ps = psum.tile([128, C], F32, tag="ps_v")
                            for ko in range(16):
                                nc.tensor.matmul(
                                    ps[:], xTr[:, ko, ts * 128:(ts + 1) * 128],
                                    w_v[:, ko, :],
                                    start=(ko == 0), stop=(ko == 15))
                            vcp = tp_.tile([128, C], BF16, tag="vcp")
                            nc.vector.tensor_copy(vcp[:], ps[:])
                            to = chk * (TCH // 128) + ts
                            nc.sync.dma_start(sp_v[b, :, to, :], vcp[:])

            # ================= work phase =================
            with ExitStack() as wk:
                wkp = wk.enter_context(tc.tile_pool(name="work", bufs=1))
                scr = wk.enter_context(tc.tile_pool(name="scr", bufs=5))
                brp = wk.enter_context(tc.tile_pool(name="brp", bufs=2))
                br1 = wk.enter_context(tc.tile_pool(name="br1", bufs=1))
                esp = wk.enter_context(tc.tile_pool(name="esp", bufs=2))
                stp = wk.enter_context(tc.tile_pool(name="stp", bufs=2))
                psA = wk.enter_context(
                    tc.tile_pool(name="psumA", bufs=1, space="PSUM"))
                psB = wk.enter_context(
                    tc.tile_pool(name="psumB", bufs=2, space="PSUM"))

                cos_t = wkp.tile([DH, T], F32, tag="cos_t")
                sin_t = wkp.tile([DH, T], F32, tag="sin_t")
                nc.sync.dma_start(cos_t[:], cos_d[:])
                nc.sync.dma_start(sin_t[:], sin_d[:])
                sgn_t = wkp.tile([DH, 1], F32, tag="sgn_t")
                nc.sync.dma_start(sgn_t[:], sgn_d[:])
                one_b = nc.const_aps.scalar_like(1.0, cos_t[:])

                wc_r = wkp.tile([128, HC, D], BF16, tag="wc_r")
                for h in range(HC):
                    wch = scr.tile([128, D], F32, tag="scr", name="wch")
                    nc.sync.dma_start(wch[:], wc_d[h * 128:(h + 1) * 128, :])
                    nc.vector.tensor_copy(wc_r[:, h, :], wch[:])

                v_r = {}
                outT = {}

                def stage_p2(b, h):
                    st = {}
                    if h == 0:
                        v_r[b] = wkp.tile([128, NTB, C], BF16, tag=f"v_r{b}",
                                          name=f"v_r{b}")
                        nc.sync.dma_start(v_r[b][:], sp_v[b])
                        outT[b] = wkp.tile([128, HC, T], BF16, tag=f"outT{b}",
                                           name=f"outT{b}")

                    brow = scr.tile([1, T], F32, tag="scr", name="brow")
                    nc.sync.dma_start(brow[:], clk_d[b, h:h + 1, :])
                    r_clk = scr.tile([128, T], F32, tag="scr", name="r_clk")
                    nc.gpsimd.partition_broadcast(r_clk[:], brow[:])
                    # clock = softplus(pre) + 1e-6, replicated per lane
                    nc.scalar.activation(r_clk[:], r_clk[:], AF.Exp)
                    nc.scalar.activation(r_clk[:], r_clk[:], AF.Ln,
                                         bias=one_b[:128])
                    nc.vector.tensor_scalar_add(r_clk[:], r_clk[:], 1e-6)
                    r_ccs = scr.tile([128, T], F32, tag="scr", name="r_ccs")
                    nc.vector.tensor_tensor_scan(r_ccs[:], r_clk[:], r_clk[:],
                                                 0.0, ALU.add, ALU.bypass)
                    nc.vector.reciprocal_approx_fast(r_ccs[:], r_ccs[:])

                    qR = scr.tile([128, T], F32, tag="scr", name="qR")
                    kR = scr.tile([128, T], F32, tag="scr", name="kR")
                    nc.sync.dma_start(qR[:], sp["q"][b, h])
                    nc.sync.dma_start(kR[:], sp["k"][b, h])
                    for t_ in (qR, kR):
                        sh = scr.tile([128, T], F32, tag="scr", name="sh")
                        nc.vector.stream_shuffle(sh[:], t_[:], SWAP32)
                        nc.vector.tensor_scalar_mul(sh[:], sh[:], sgn_t[:, 0:1])
                        nc.vector.tensor_mul(sh[:], sh[:], sin_t[:])
                        nc.vector.tensor_mul(t_[:], t_[:], cos_t[:])
                        nc.vector.tensor_add(t_[:], t_[:], sh[:])

                    q1 = brp.tile([128, T], F32R, tag="br_q1", name="q1")
                    k1 = brp.tile([128, T], F32R, tag="br_k1", name="k1")
                    q3 = brp.tile([128, T], F32R, tag="br_q3", name="q3")
                    k3 = brp.tile([128, T], F32R, tag="br_k3", name="k3")
                    q2h = br1.tile([128, T], BF16, tag="br_q2h", name="q2h")
                    q2l = br1.tile([128, T], BF16, tag="br_q2l", name="q2l")
                    k2h = br1.tile([128, T], BF16, tag="br_k2h", name="k2h")
                    k2l = br1.tile([128, T], BF16, tag="br_k2l", name="k2l")

                    # branch 3 first (frees r_ccs)
                    nc.vector.tensor_mul(q3[:], qR[:], r_ccs[:])
                    nc.vector.tensor_mul(k3[:], kR[:], r_clk[:])

                    # branch 1
                    r_lclk = scr.tile([128, T], F32, tag="scr", name="r_lclk")
                    nc.scalar.activation(r_lclk[:], r_clk[:], AF.Ln)
                    pT = scr.tile([128, T], F32, tag="scr", name="pT")
                    nc.sync.dma_start(pT[:], sp["p"][b, h])
                    nc.vector.tensor_add(pT[:], pT[:], r_lclk[:])
                    nmx1 = wkp.tile([128, 1], F32, tag="pmax")
                    nc.vector.tensor_reduce(nmx1[:], pT[:], axis=AX.X,
                                            op=ALU.max, negate=True)
                    nc.scalar.activation(pT[:], pT[:], AF.Exp,
                                         bias=nmx1[:, 0:1])  # p_exp
                    pcs = scr.tile([128, T], F32, tag="scr", name="pcs")
                    nc.vector.tensor_tensor_scan(pcs[:], pT[:], pT[:],
                                                 0.0, ALU.add, ALU.bypass)
                    nc.vector.tensor_scalar_add(pcs[:], pcs[:], 1e-8)
                    nc.vector.reciprocal_approx_fast(pcs[:], pcs[:])
                    nc.vector.tensor_mul(q1[:], qR[:], pcs[:])
                    nc.vector.tensor_mul(k1[:], kR[:], pT[:])

                    # branch 2
                    gT = scr.tile([128, T], F32, tag="scr", name="gT")
                    nc.sync.dma_start(gT[:], sp["g"][b, h])
                    nc.scalar.activation(gT[:], gT[:], AF.Exp)
                    nc.scalar.activation(gT[:], gT[:], AF.Ln, bias=one_b[:128])
                    nc.vector.tensor_mul(gT[:], gT[:], r_clk[:])
                    nc.vector.tensor_scalar_mul(gT[:], gT[:], -1.0)
                    gcs = scr.tile([128, T], F32, tag="scr", name="gcs")
                    nc.vector.tensor_tensor_scan(gcs[:], gT[:], gT[:],
                                                 0.0, ALU.add, ALU.bypass)
                    nc.vector.tensor_scalar(gcs[:], gcs[:], 40.0, -50.0,
                                            ALU.min, ALU.max)
                    nc.scalar.activation(gcs[:], gcs[:], AF.Exp)  # gj_cp
                    q2 = scr.tile([128, T], F32, tag="scr", name="q2")
                    nc.vector.tensor_mul(q2[:], qR[:], gcs[:])
                    nc.vector.tensor_copy(q2h[:], q2[:])
                    nc.vector.tensor_sub(q2l[:], q2[:], q2h[:])
                    nc.vector.tensor_scalar_add(gcs[:], gcs[:], 1e-8)
                    nc.vector.reciprocal_approx_fast(gcs[:], gcs[:])
                    k2 = scr.tile([128, T], F32, tag="scr", name="k2")
                    nc.vector.tensor_mul(k2[:], kR[:], gcs[:])
                    nc.vector.tensor_copy(k2h[:], k2[:])
                    nc.vector.tensor_sub(k2l[:], k2[:], k2h[:])
                    st.update(q1=q1, k1=k1, q3=q3, k3=k3,
                              q2h=q2h, q2l=q2l, k2h=k2h, k2l=k2l)
                    return st

                def stage_p3(b, h, st):
                    q1, k1 = st["q1"], st["k1"]
                    q3, k3 = st["q3"], st["k3"]
                    q2h, q2l = st["q2h"], st["q2l"]
                    k2h, k2l = st["k2h"], st["k2l"]
                    for tg in range(8):  # groups of 2 t-blocks
                        eT2 = wkp.tile([128, NTB, 256], BF16, tag="eT2",
                                       name="eT2")
                        for tj in range(2):
                            tb = tg * 2 + tj
                            tsl = slice(tb * 128, (tb + 1) * 128)
                            pS = [psA.tile([128, 512], F32, tag=f"ps_S{i}",
                                           name=f"ps_S{i}")
                                  for i in range(4)]
                            # branch-major: each lhsT (weights) loads once
                            # and streams >=2048 columns before switching
                            for i in range(4):
                                nc.tensor.matmul(pS[i][:], q1[:, tsl],
                                                 k1[:, i * 512:(i + 1) * 512],
                                                 start=True, stop=False)
                            for kk in (k2h, k2l):
                                for i in range(4):
                                    nc.tensor.matmul(
                                        pS[i][:], q2h[:, tsl],
                                        kk[:, i * 512:(i + 1) * 512],
                                        start=False, stop=False,
                                        skip_group_check=True)
                            for i in range(4):
                                nc.tensor.matmul(pS[i][:], q2l[:, tsl],
                                                 k2h[:, i * 512:(i + 1) * 512],
                                                 start=False, stop=False,
                                                 skip_group_check=True)
                            for i in range(4):
                                nc.tensor.matmul(pS[i][:], q3[:, tsl],
                                                 k3[:, i * 512:(i + 1) * 512],
                                                 start=False, stop=True,
                                                 skip_group_check=True)
                            # online softmax: per-group local max + exp,
                            # then per-group rescale after global max
                            nmx4 = stp.tile([128, 4], F32, tag="nmx4")
                            bs4 = stp.tile([128, 4], F32, tag="bs4")
                            eS = esp.tile([128, T], BF16, tag="eS")
                            sm4 = stp.tile([128, 4], F32, tag="sm4")
                            for i in range(4):
                                ssl = slice(i * 512, (i + 1) * 512)
                                nc.vector.tensor_reduce(
                                    nmx4[:, i:i + 1], pS[i][:],
                                    axis=AX.X, op=ALU.max, negate=True)
                                nc.vector.tensor_scalar_mul(
                                    bs4[:, i:i + 1], nmx4[:, i:i + 1], SCALE)
                                nc.scalar.activation(
                                    eS[:, ssl], pS[i][:], AF.Exp,
                                    bias=bs4[:, i:i + 1], scale=SCALE,
                                    accum_out=sm4[:, i:i + 1])
                            # gmax = max_i lmax_i = -min_i nmx4_i
                            gmx = stp.tile([128, 1], F32, tag="gmx")
                            nc.vector.tensor_reduce(gmx[:], nmx4[:],
                                                    axis=AX.X, op=ALU.min,
                                                    negate=True)
                            d4 = stp.tile([128, 4], F32, tag="d4")
                            nc.vector.tensor_scalar_add(d4[:], nmx4[:],
                                                        gmx[:, 0:1])
                            f4 = stp.tile([128, 4], F32, tag="f4")
                            nc.scalar.activation(f4[:], d4[:], AF.Exp,
                                                 scale=-SCALE)
                            wsm = stp.tile([128, 4], F32, tag="wsm")
                            nc.vector.tensor_mul(wsm[:], sm4[:], f4[:])
                            rs = stp.tile([128, 1], F32, tag="rs")
                            nc.vector.tensor_reduce(rs[:], wsm[:],
                                                    axis=AX.X, op=ALU.add)
                            rrs = stp.tile([128, 1], F32, tag="rrs")
                            nc.vector.reciprocal_approx_fast(rrs[:], rs[:])
                            fs4 = stp.tile([128, 4], F32, tag="fs4")
                            nc.vector.tensor_scalar_mul(fs4[:], f4[:],
                                                        rrs[:, 0:1])
                            for sg in range(4):
                                ssl = slice(sg * 512, (sg + 1) * 512)
                                nc.scalar.mul(eS[:, ssl], eS[:, ssl],
                                              fs4[:, sg:sg + 1])
                                pstr = psB.tile([128, 512], BF16,
                                                tag="ps_eT")
                                for j in range(4):
                                    so = sg * 4 + j
                                    nc.tensor.transpose(
                                        pstr[:, j * 128:(j + 1) * 128],
                                        eS[:, so * 128:(so + 1) * 128],
                                        id_b[:])
                                nc.vector.tensor_copy(
                                    eT2[:, sg * 4:(sg + 1) * 4,
                                        tj * 128:(tj + 1) * 128],
                                    pstr[:].rearrange("p (a b) -> p a b", a=4))
                        pav = psA.tile([128, 256], F32, tag="ps_av")
                        csl = slice(h * DH, (h + 1) * DH)
                        for so in range(NTB):
                            nc.tensor.matmul(pav[:], v_r[b][:, so, csl],
                                             eT2[:, so, :],
                                             start=(so == 0), stop=(so == 15))
                        nc.vector.tensor_copy(
                            outT[b][:, h, tg * 256:(tg + 1) * 256], pav[:])

                def stage_p4(b):
                    for tb in range(NTB):
                        tsl = slice(tb * 128, (tb + 1) * 128)
                        fin = esp.tile([128, D], BF16, tag="fin", name="fin")
                        for nk in range(4):
                            nsl = slice(nk * 512, (nk + 1) * 512)
                            psf = psA.tile([128, 512], F32, tag="ps_fin")
                            for h in range(HC):
                                nc.tensor.matmul(psf[:], outT[b][:, h, tsl],
                                                 wc_r[:, h, nsl],
                                                 start=(h == 0), stop=(h == 1))
                            nc.scalar.copy(fin[:, nsl], psf[:])
                        nc.sync.dma_start(out_d[b, tsl, :], fin[:])

                pairs = [(b, h) for b in range(B) for h in range(HC)]
                prev = None
                for i, (b, h) in enumerate(pairs):
                    st = stage_p2(b, h)
                    if prev is not None:
                        pb, ph, pst = prev
                        stage_p3(pb, ph, pst)
                        if ph == HC - 1:
                            stage_p4(pb)
                    prev = (b, h, st)
                pb, ph, pst = prev
                stage_p3(pb, ph, pst)
                stage_p4(pb)

    nc.compile()
    return nc


_PROGRAM_CACHE = None


def _get_program():
    global _PROGRAM_CACHE
    if _PROGRAM_CACHE is None:
        _PROGRAM_CACHE = _build_program()
    return _PROGRAM_CACHE


def _host_tables():
    d = DH
    inv_freq = 1.0 / (np.float32(10000.0) **
                      (np.arange(0, d, 2, dtype=np.float32) / np.float32(d)))
    t = np.arange(T, dtype=np.float32)
    freqs = t[:, None] * inv_freq[None, :].astype(np.float32)
    emb = np.concatenate([freqs, freqs], axis=-1).astype(np.float32)
    cosT = np.ascontiguousarray(np.cos(emb).astype(np.float32).T)
    sinT = np.ascontiguousarray(np.sin(emb).astype(np.float32).T)
    sgn = np.where(np.arange(d) % 2 == 0, -1.0, 1.0).astype(np.float32)[:, None]
    return cosT, sinT, np.ascontiguousarray(sgn)


def kernel(x, W_q, W_k, W_v, W_gate, W_p, W_clock, W_c, _trace=False,
           _core_ids=None):
    x = np.ascontiguousarray(np.asarray(x, dtype=np.float32))
    cosT, sinT, sgn = _host_tables()
    ident = np.eye(128, dtype=np.float32)
    core_ids = list(range(NCORES)) if _core_ids is None else list(_core_ids)

    in_maps = []
    for c in core_ids:
        c0 = c * C
        wg_ext = np.concatenate(
            [np.asarray(W_gate)[:, c0:c0 + C],
             np.asarray(W_clock)[:, c * HC:(c + 1) * HC]], axis=1)
        in_maps.append({
            "x": x,
            "wq": np.ascontiguousarray(np.asarray(W_q)[:, c0:c0 + C]),
            "wk": np.ascontiguousarray(np.asarray(W_k)[:, c0:c0 + C]),
            "wg": np.ascontiguousarray(wg_ext.astype(np.float32)),
            "wp": np.ascontiguousarray(np.asarray(W_p)[:, c0:c0 + C]),
            "wv": np.ascontiguousarray(np.asarray(W_v)[:, c0:c0 + C]),
            "wc": np.ascontiguousarray(np.asarray(W_c)[c0:c0 + C, :]),
            "cosT": cosT, "sinT": sinT, "sgn": sgn, "ident": ident,
        })

    nc = _get_program()
    res = run_bass_kernel_spmd(nc, in_maps, core_ids=core_ids, trace=_trace)
    out = np.zeros((B, T, D), dtype=np.float64)
    for r in res.results:
        out += r["out"].astype(np.float64)
    kernel._last_result = res
    return out.astype(np.float32)
